# revision 12
# baseline (speedup 1.0000x reference)
"""Trainium2 Bass kernel for nn_MultiHeadAttention (B=4, S=2048, D=1024, H=16).

Sharding: 8 cores = 4 batches x 2 row-halves. Core (b, h) receives ONLY its
1024 rows of batch b (these serve as both its queries and its key-half). It
computes Q projections for its rows and K/V projections for its key-half;
the full K/V (all 2048 keys) is assembled with a pairwise AllGather
(replica groups [2b, 2b+1]; rank 0 holds rows 0-1023, so the gathered
chunks land in canonical key order on both cores). This halves the K/V
projection and transpose work versus computing full K/V per core.

Per-core dataflow (bf16 matmuls, fp32 PSUM accumulation):
  xh [1024,1024] --PE transpose--> xT [D, 1024]
  kT-own / v-own for the 1024 own keys (all 8 head pairs), staged to
  internal DRAM, AllGather, read back as kp [d-pair, 2048] and
  v_aug = [x @ Wv | ones] for all 16 key chunks.
  bk is dropped (its score offset is constant over keys -> cancels in
  softmax); bv contributes bv@Wo to y (sum(attn)=1) and is folded into bo
  on the host along with nothing else; bq is kept on the Q side.
  Per head pair p, per q-span of 512:
    scoresT[k,q] via row-paired (tile_position) K=64 matmuls
    exp on ACT (scale=1/8 folded in), table pre-warmed at kernel start
    attnV with M=65 aug (softmax denominator rides row 64 of PSUM)
    split epilogue: denominators and raw rows staged out of PSUM at once;
    reciprocal + K=1 broadcast matmuls + in-place normalize of oT deferred
    into the next span so the PE never waits on the DVE chain.
  y = oT^T @ Wo + bo'  (bo' = bo + bv@Wo precomputed on host)
  O-projection for q-span 0 interleaved into pair 7 span 1.
"""

import numpy as np
import ml_dtypes
from contextlib import ExitStack

import concourse.bass as bass
from concourse import bacc
import concourse.mybir as mybir
import concourse.tile as tile
from concourse.bass_utils import run_bass_kernel_spmd
from concourse.masks import make_identity

F32 = mybir.dt.float32
BF16 = mybir.dt.bfloat16
AF = mybir.ActivationFunctionType
NPBF16 = ml_dtypes.bfloat16

P = 128

N_CORES = 8
B_FULL, S_FULL, D_FULL = 4, 2048, 1024
H_FULL, DH = 16, 64
GROUPS = [[0, 1], [2, 3], [4, 5], [6, 7]]


def build_mha_nc(S=2048, Sq=1024, D=1024, H=16, scale=None):
    """Build the per-core Bass program. Returns nc."""
    assert D % P == 0 and S % P == 0 and Sq % P == 0 and H % 2 == 0
    ND = D // P            # d-tiles
    NS = S // P            # k-tiles over the full (gathered) key axis
    NSO = Sq // P          # own key chunks
    NPAIR = H // 2
    W65 = DH + 1           # augmented head width (v | ones)
    QSP = min(512, Sq)     # q span
    NQS = Sq // QSP
    KSP = 512              # span for own-kT projection (2 spans = 1024 keys)
    NKSO = Sq // KSP
    CSP = min(512, D)      # col span for v / out projections
    NCS = D // CSP
    HPS = CSP // DH        # heads per col-span in v projection
    if scale is None:
        scale = DH ** -0.5
    KV_K = NPAIR * P * Sq          # own kT elements
    KV_V = P * NSO * H * W65       # own v elements (aug column included)
    KV_E = KV_K + KV_V

    nc = bacc.Bacc(target_bir_lowering=False, debug=False, num_devices=N_CORES)

    x = nc.dram_tensor("x", [Sq, D], BF16, kind="ExternalInput").ap()
    W = {n: nc.dram_tensor(n, [D, D], BF16, kind="ExternalInput").ap()
         for n in ("Wq", "Wk", "Wv", "Wo")}
    bias = {n: nc.dram_tensor(n, [D], F32, kind="ExternalInput").ap()
            for n in ("bq", "bo")}
    ones_d = nc.dram_tensor("cst_ones", [P, P], BF16, kind="ExternalInput").ap()
    y = nc.dram_tensor("y", [Sq, D], F32, kind="ExternalOutput").ap()
    kv_out = nc.dram_tensor("kv_out", [KV_E], BF16).ap()
    kv_all = nc.dram_tensor("kv_all", [2 * KV_E], BF16).ap()

    with tile.TileContext(nc) as tc, ExitStack() as top:
        top.enter_context(nc.allow_low_precision(
            reason="bf16 activations/weights with fp32 psum accumulation"))
        const = top.enter_context(tc.tile_pool(name="const", bufs=1))
        big = top.enter_context(tc.tile_pool(name="big", bufs=1))
        wp = top.enter_context(tc.tile_pool(name="wp", bufs=2))

        ident = const.tile([P, P], BF16)
        make_identity(nc, ident)
        # bf16 ones row: K=1 stationary broadcasting the softmax reciprocal
        ones_t = const.tile([1, DH], BF16)
        nc.vector.memset(ones_t, 1.0)
        # warm the ACT exp table while DMAs run
        warm = const.tile([1, 2], BF16)
        nc.scalar.activation(warm, ones_t[:, 0:2], AF.Exp, scale=1.0)

        # per-partition bias layouts: b_sb[p, j] = b[j*128 + p]
        bq_sb = const.tile([P, ND], F32)
        nc.gpsimd.dma_start(out=bq_sb, in_=bias["bq"].rearrange("(j p) -> p j", p=P))
        # bo broadcast across partitions (0-stride DRAM read)
        bo_bc = const.tile([P, D], F32)
        nc.gpsimd.dma_start(
            out=bo_bc,
            in_=bias["bo"].unsqueeze(0).partition_broadcast(P).squeeze(1),
        )

        oT = big.tile([P, ND, Sq], BF16)
        xT = big.tile([P, ND, Sq], BF16)
        qTs = big.tile([P, ND, Sq], BF16)
        v_sb = big.tile([P, NS, H * W65], BF16)
        v3 = v_sb.rearrange("p i (h w) -> p i h w", w=W65)
        kp = big.tile([P, 2, NPAIR, Sq], BF16)

        # weight staging (wp "w" rotates: Wk, Wq, then Wo; Wv pinned)
        Wk_sb = wp.tile([P, ND, D], BF16, tag="w", name="Wk")
        Wq_sb = wp.tile([P, ND, D], BF16, tag="w", name="Wq")
        Wv_sb = wp.tile([P, ND, D], BF16, tag="wv", bufs=1)

        wo_box = {}

        # ---- prologue: own-half transposes + K/V projections + AllGather.
        # Its PSUM pools close before the attention pools open. ----
        with tc.tile_pool(name="xchunk", bufs=1) as xpool, \
             tc.tile_pool(name="tps", bufs=2, space="PSUM") as tpsum, \
             tc.tile_pool(name="ppE", bufs=3, space="PSUM") as ppE:
            xc = xpool.tile([P, NSO, D], BF16)
            for i in range(NSO):
                nc.sync.dma_start(out=xc[:, i, :], in_=x[i * P:(i + 1) * P, :])
            nc.sync.dma_start(
                out=Wk_sb, in_=W["Wk"].rearrange("(j p) c -> p j c", p=P))
            nc.sync.dma_start(
                out=Wv_sb, in_=W["Wv"].rearrange("(j p) c -> p j c", p=P))
            nc.sync.dma_start(
                out=Wq_sb, in_=W["Wq"].rearrange("(j p) c -> p j c", p=P))
            for i in range(NSO):
                nc.sync.dma_start(out=v3[:, i, :, DH:DH + 1],
                                  in_=ones_d[:, 0:H].unsqueeze(2))
            for i in range(NSO):
                for j in range(ND):
                    tp = tpsum.tile([P, P], BF16, tag="tp")
                    nc.tensor.transpose(tp, xc[:, i, j * P:(j + 1) * P], ident)
                    nc.vector.tensor_copy(xT[:, j, i * P:(i + 1) * P], tp)
            # own-half kT for all pairs -> kp[:, p, 0:1024] (staging; the
            # gather readback rewrites the full key axis in canonical order)
            for p_ in range(NPAIR):
                for sp_ in range(NKSO):
                    ps = ppE.tile([P, KSP], F32, tag="pp", name=f"k_{p_}_{sp_}")
                    for j in range(ND):
                        nc.tensor.matmul(
                            ps,
                            Wk_sb[:, j, p_ * P:(p_ + 1) * P],
                            xT[:, j, sp_ * KSP:(sp_ + 1) * KSP],
                            start=(j == 0), stop=(j == ND - 1),
                        )
                    nc.vector.tensor_copy(
                        kp[:, 0, p_, sp_ * KSP:(sp_ + 1) * KSP], ps)
            # own-half v -> v3[:, 0:8, :, :]
            for i in range(NSO):
                for sp_ in range(NCS):
                    ps = ppE.tile([P, CSP], F32, tag="pp", name=f"v_{i}_{sp_}")
                    for j in range(ND):
                        nc.tensor.matmul(
                            ps,
                            xT[:, j, i * P:(i + 1) * P],
                            Wv_sb[:, j, sp_ * CSP:(sp_ + 1) * CSP],
                            start=(j == 0), stop=(j == ND - 1),
                        )
                    nc.vector.tensor_copy(
                        v3[:, i, sp_ * HPS:(sp_ + 1) * HPS, 0:DH],
                        ps.rearrange("p (h w) -> p h w", w=DH),
                    )
            # stage own halves to DRAM, gather, read back both chunks
            nc.sync.dma_start(
                out=kv_out[0:KV_K].rearrange(
                    "(p pr k) -> p pr k", p=P, pr=NPAIR),
                in_=kp[:, 0, :, :],
            )
            nc.sync.dma_start(
                out=kv_out[KV_K:KV_E].rearrange(
                    "(p i hw) -> p i hw", p=P, i=NSO),
                in_=v_sb[:, 0:NSO, :],
            )
            nc.gpsimd.collective_compute(
                "AllGather",
                mybir.AluOpType.bypass,
                replica_groups=GROUPS,
                ins=[kv_out],
                outs=[kv_all],
            )
            kvc = kv_all.rearrange("(c e) -> c e", c=2)
            for c in range(2):
                nc.sync.dma_start(
                    out=kp[:, c, :, :],
                    in_=kvc[c, 0:KV_K].rearrange("(p pr k) -> p pr k",
                                                 p=P, pr=NPAIR),
                )
                nc.sync.dma_start(
                    out=v_sb[:, c * NSO:(c + 1) * NSO, :],
                    in_=kvc[c, KV_K:KV_E].rearrange("(p i hw) -> p i hw",
                                                    p=P, i=NSO),
                )

        with tc.tile_pool(name="exp", bufs=4) as exq, \
             tc.tile_pool(name="eps", bufs=2) as eps, \
             tc.tile_pool(name="scps", bufs=2, space="PSUM") as scps, \
             tc.tile_pool(name="ystg", bufs=2) as ystg, \
             tc.tile_pool(name="pps", bufs=2, space="PSUM") as pps, \
             tc.tile_pool(name="ops", bufs=2, space="PSUM") as opsum:

            def qT_proj(dc, sp):
                ps = pps.tile([P, QSP], F32, tag="pp", name=f"qps_{dc}_{sp}")
                for j in range(ND):
                    nc.tensor.matmul(
                        ps,
                        Wq_sb[:, j, dc * P:(dc + 1) * P],
                        xT[:, j, sp * QSP:(sp + 1) * QSP],
                        start=(j == 0), stop=(j == ND - 1),
                    )
                nc.vector.tensor_scalar_add(
                    qTs[:, dc, sp * QSP:(sp + 1) * QSP], ps,
                    bq_sb[:, dc:dc + 1])

            def load_wo():
                Wo_sb = wp.tile([P, ND, D], BF16, tag="w", name="Wo")
                nc.sync.dma_start(
                    out=Wo_sb, in_=W["Wo"].rearrange("(j p) c -> p j c", p=P))
                wo_box["Wo"] = Wo_sb

            def o_chunk(sc_i, spc):
                Wo_sb = wo_box["Wo"]
                ps = pps.tile([P, CSP], F32, tag="pp",
                              name=f"yps_{sc_i}_{spc}")
                for j in range(ND):
                    nc.tensor.matmul(
                        ps,
                        oT[:, j, sc_i * P:(sc_i + 1) * P],
                        Wo_sb[:, j, spc * CSP:(spc + 1) * CSP],
                        start=(j == 0), stop=(j == ND - 1),
                    )
                ysb = ystg.tile([P, CSP], F32, tag="ysb")
                nc.vector.tensor_add(
                    ysb, ps, bo_bc[:, spc * CSP:(spc + 1) * CSP])
                nc.sync.dma_start(
                    out=y[sc_i * P:(sc_i + 1) * P,
                          spc * CSP:(spc + 1) * CSP],
                    in_=ysb,
                )

            # queries for the first pairs; overlaps the gather latency
            for dc in (0, 1, 2):
                for sp_ in range(NQS):
                    qT_proj(dc, sp_)
            load_wo()

            # deferred-work schedule: (pair, span, kt) -> [thunks]
            jobs = {}

            def add(p, sp, kt, fn):
                jobs.setdefault((p, sp, kt), []).append(fn)

            for dc in range(3, ND):
                add(dc - 2, 0, 5, lambda dc=dc: qT_proj(dc, 0))
                add(dc - 2, 0, 10, lambda dc=dc: qT_proj(dc, 1))
            # O-projection span 0 interleaved into pair 7 span 1; slots
            # start after the deferred epilogue of (7, 0) fires at kt==2
            for (sc_i, spc), kt_ in zip(
                    [(si, c) for si in range(QSP // P) for c in range(NCS)],
                    (3, 4, 6, 8, 10, 12, 14, 15)):
                add(NPAIR - 1, 1, kt_, lambda a=sc_i, b=spc: o_chunk(a, b))

            # deferred epilogue part 2: reciprocal + broadcast + in-place
            # normalize of oT; runs inside the NEXT span's kt loop
            def epi_b(p, sp, den):
                nc.vector.reciprocal_approx_fast(den, den)
                rc16 = eps.tile([1, 2 * QSP], BF16, tag="rc16")
                nc.vector.tensor_copy(rc16, den)
                qsl = slice(sp * QSP, (sp + 1) * QSP)
                rb_ps = pps.tile([P, QSP], F32, tag="pp",
                                 name=f"rb_{p}_{sp}")
                nc.tensor.matmul(
                    rb_ps[0:DH, :], ones_t, rc16[:, 0:QSP],
                    start=True, stop=True,
                )
                nc.tensor.matmul(
                    rb_ps[DH:P, :], ones_t, rc16[:, QSP:2 * QSP],
                    start=True, stop=True,
                )
                rb = eps.tile([P, QSP], F32, tag="rb")
                nc.vector.tensor_copy(rb, rb_ps)
                nc.vector.tensor_mul(oT[:, p, qsl], oT[:, p, qsl], rb)

            pending = []

            # ---- attention: pair-outer, span-inner ----
            for p in range(NPAIR):
                for sp in range(NQS):
                    qsl = slice(sp * QSP, (sp + 1) * QSP)
                    o_even = opsum.tile([W65, QSP], F32, tag="op")
                    o_odd = opsum.tile([W65, QSP], F32, tag="op")
                    for kt in range(NS):
                        if kt == 2 and pending:
                            pending.pop()()
                        for fn in jobs.get((p, sp, kt), ()):
                            fn()
                        sc = scps.tile([P, 2 * QSP], F32, tag="sc")
                        kc, ko = divmod(kt, NSO)
                        nc.tensor.matmul(
                            sc[:, 0:QSP],
                            kp[0:DH, kc, p, ko * P:(ko + 1) * P],
                            qTs[0:DH, p, qsl],
                            start=True, stop=True,
                        )
                        nc.tensor.matmul(
                            sc[:, QSP:2 * QSP],
                            kp[DH:P, kc, p, ko * P:(ko + 1) * P],
                            qTs[DH:P, p, qsl],
                            start=True, stop=True,
                        )
                        ex = exq.tile([P, 2 * QSP], BF16, tag="ex")
                        nc.scalar.activation(ex, sc, AF.Exp,
                                             scale=float(scale))
                        nc.tensor.matmul(
                            o_even,
                            v3[:, kt, 2 * p, :],
                            ex[:, 0:QSP],
                            start=(kt == 0), stop=(kt == NS - 1),
                        )
                        nc.tensor.matmul(
                            o_odd,
                            v3[:, kt, 2 * p + 1, :],
                            ex[:, QSP:2 * QSP],
                            start=(kt == 0), stop=(kt == NS - 1),
                        )
                    # epilogue part 1: stage denominators and raw rows out
                    # of PSUM so the accumulators free quickly
                    den = eps.tile([1, 2 * QSP], F32, tag="den")
                    nc.vector.tensor_copy(den[:, 0:QSP], o_even[DH:W65, :])
                    nc.vector.tensor_copy(den[:, QSP:2 * QSP],
                                          o_odd[DH:W65, :])
                    nc.vector.tensor_copy(oT[0:DH, p, qsl], o_even[0:DH, :])
                    nc.vector.tensor_copy(oT[DH:P, p, qsl], o_odd[0:DH, :])
                    pending.append(
                        lambda p=p, sp=sp, den=den: epi_b(p, sp, den))
            # flush the last deferred epilogue, then tail O-projection
            while pending:
                pending.pop()()
            for sc_i in range(QSP // P, Sq // P):
                for spc in range(NCS):
                    o_chunk(sc_i, spc)

    nc.compile()
    return nc


_NC = None


def _get_nc():
    global _NC
    if _NC is None:
        _NC = build_mha_nc(S=S_FULL, Sq=S_FULL // 2, D=D_FULL, H=H_FULL)
    return _NC


def shard_inputs(inputs):
    x = np.asarray(inputs["x"], dtype=np.float32).astype(NPBF16)
    wnames = ("Wq", "Wk", "Wv", "Wo")
    shared = {n: np.ascontiguousarray(
        np.asarray(inputs[n], dtype=np.float32).astype(NPBF16)) for n in wnames}
    shared["bq"] = np.ascontiguousarray(np.asarray(inputs["bq"], dtype=np.float32))
    # bv contributes bv @ Wo to y (attention rows sum to 1); fold into bo
    bv = np.asarray(inputs["bv"], dtype=np.float32)
    Wo = np.asarray(inputs["Wo"], dtype=np.float32)
    bo = np.asarray(inputs["bo"], dtype=np.float32)
    shared["bo"] = np.ascontiguousarray(bo + bv @ Wo)
    shared["cst_ones"] = np.ones((P, P), dtype=NPBF16)
    half = S_FULL // 2
    maps = []
    for c in range(N_CORES):
        b, h = divmod(c, 2)
        m = dict(shared)
        m["x"] = np.ascontiguousarray(x[b, h * half:(h + 1) * half])
        maps.append(m)
    return maps


def run(inputs, trace=False):
    nc = _get_nc()
    maps = shard_inputs(inputs)
    res = run_bass_kernel_spmd(nc, maps, list(range(N_CORES)), trace=trace)
    half = S_FULL // 2
    y = np.empty((B_FULL, S_FULL, D_FULL), dtype=np.float32)
    for c in range(N_CORES):
        b, h = divmod(c, 2)
        y[b, h * half:(h + 1) * half] = res.results[c]["y"]
    return y, res


def kernel(**inputs):
    y, _ = run(inputs, trace=False)
    return y


# revision 19
# speedup vs baseline: 1.2535x; 1.2535x over previous
"""Trainium2 Bass kernel for nn_MultiHeadAttention (B=4, S=2048, D=1024, H=16).

Sharding: 8 cores = 4 batches x 2 query-halves. Each core computes full K/V
projections for its batch (keys are permuted so the core's own queries come
first), attention for its 1024 queries over all 2048 keys, and the output
projection for its query half. No collectives needed.

Per-core dataflow (bf16 matmuls, fp32 PSUM accumulation, all tensors SBUF
resident — no DRAM spills):
  x [2048,1024] bf16 --PE transpose--> xT [D,S]
  qT = (x @ Wq)^T [D,1024] and v_aug = [x @ Wv | ones] computed up front;
  kT d-tiles are projected per head-pair, interleaved into the attention
  loop so the PE fills the gaps of the ACT(exp)-paced inner loop.
  Per head pair p, per q-span of 512:
    scoresT[k,q] tiles via row-paired (tile_position) K=64 matmuls
    exp on ACT (scale=1/8 folded in), flash-style, no max subtraction
    outT[65,q] accumulated in PSUM via v_aug=[v_h | ones] stationary
    normalize by row 64 (denominator via reciprocal_approx_fast +
    K=1 fp32 PE broadcast), add bv, assemble oT [D, Sq]
  y = oT^T @ Wo + bo  -> [1024, 1024] fp32
"""

import os
import numpy as np
import ml_dtypes
from contextlib import ExitStack

import concourse.bass as bass
from concourse import bacc
import concourse.mybir as mybir
import concourse.tile as tile
from concourse.bass_utils import run_bass_kernel_spmd
from concourse.masks import make_identity

F32 = mybir.dt.float32
BF16 = mybir.dt.bfloat16
AF = mybir.ActivationFunctionType
NPBF16 = ml_dtypes.bfloat16

P = 128

N_CORES = 8
B_FULL, S_FULL, D_FULL = 4, 2048, 1024
H_FULL, DH = 16, 64


def build_mha_nc(S=2048, Sq=1024, D=1024, H=16, scale=None):
    """Build the per-core Bass program. Returns nc."""
    assert D % P == 0 and S % P == 0 and Sq % P == 0 and H % 2 == 0
    ND = D // P            # d-tiles
    NS = S // P            # s-chunks / k-tiles
    NPAIR = H // 2
    W65 = DH + 1           # augmented head width (v | ones)
    QSP = min(512, Sq)     # q span
    NQS = Sq // QSP
    KSP = min(512, S)      # span for kT projection
    NKS = S // KSP
    CSP = min(512, D)      # col span for v / out projections
    NCS = D // CSP
    HPS = CSP // DH        # heads per col-span in v projection
    if scale is None:
        scale = DH ** -0.5

    nc = bacc.Bacc(target_bir_lowering=False, debug=False)

    x = nc.dram_tensor("x", [S, D], BF16, kind="ExternalInput").ap()
    W = {n: nc.dram_tensor(n, [D, D], BF16, kind="ExternalInput").ap()
         for n in ("Wq", "Wk", "Wv", "Wo")}
    bias = {n: nc.dram_tensor(n, [D], F32, kind="ExternalInput").ap()
            for n in ("bq", "bo")}
    ones_d = nc.dram_tensor("cst_ones", [P, P], BF16, kind="ExternalInput").ap()
    y = nc.dram_tensor("y", [Sq, D], F32, kind="ExternalOutput").ap()

    with tile.TileContext(nc) as tc, ExitStack() as top:
        top.enter_context(nc.allow_low_precision(
            reason="bf16 activations/weights with fp32 psum accumulation"))
        const = top.enter_context(tc.tile_pool(name="const", bufs=1))
        big = top.enter_context(tc.tile_pool(name="big", bufs=1))
        wp = top.enter_context(tc.tile_pool(name="wp", bufs=2))
        kpool = top.enter_context(tc.tile_pool(name="kpool", bufs=3))
        ppsL = top.enter_context(tc.tile_pool(name="ppsL", bufs=1, space="PSUM"))

        ident = const.tile([P, P], BF16)
        make_identity(nc, ident)
        # bf16 ones row: K=1 stationary broadcasting the softmax denominator
        ones_t = const.tile([1, DH], BF16)
        nc.vector.memset(ones_t, 1.0)
        # warm the ACT exp table while the first DMAs run
        warm = const.tile([1, 2], BF16)
        nc.scalar.activation(warm, ones_t[:, 0:2], AF.Exp, scale=1.0)

        # per-partition bias layouts: b_sb[p, j] = b[j*128 + p]
        # (bk dropped: its q.bk score offset is constant over keys and
        # cancels in softmax; bv folded into bo on the host as bv @ Wo)
        bq_sb = const.tile([P, ND], F32)
        nc.gpsimd.dma_start(out=bq_sb, in_=bias["bq"].rearrange("(j p) -> p j", p=P))
        # bo broadcast across partitions (0-stride DRAM read)
        bo_bc = const.tile([P, D], F32)
        nc.gpsimd.dma_start(
            out=bo_bc,
            in_=bias["bo"].unsqueeze(0).partition_broadcast(P).squeeze(1),
        )

        oT = big.tile([P, ND, Sq], BF16)
        xT = big.tile([P, ND, S], BF16)
        qTs = big.tile([P, ND, Sq], BF16)
        v_sb = big.tile([P, NS, H * W65], BF16)

        # ---- Phase T: PE-transpose x into xT ----
        with tc.tile_pool(name="xchunk", bufs=3) as xpool, \
             tc.tile_pool(name="tps", bufs=2, space="PSUM") as tpsum, \
             tc.tile_pool(name="ppsE", bufs=4, space="PSUM") as ppsE:
            for i in range(NS):
                xc = xpool.tile([P, D], BF16, tag="xc")
                nc.sync.dma_start(out=xc, in_=x[i * P:(i + 1) * P, :])
                for j in range(ND):
                    tp = tpsum.tile([P, P], BF16, tag="tp")
                    nc.tensor.transpose(tp, xc[:, j * P:(j + 1) * P], ident)
                    nc.vector.tensor_copy(xT[:, j, i * P:(i + 1) * P], tp)

            # ---- qT / v projections: helpers; early part emits only what
            # pair 0 needs, the rest interleaves into the attention loop ----
            Wq_sb = wp.tile([P, ND, D], BF16, tag="w")
            nc.sync.dma_start(out=Wq_sb, in_=W["Wq"].rearrange("(j p) c -> p j c", p=P))
            Wv_sb = wp.tile([P, ND, D], BF16, tag="wv", bufs=1)
            nc.sync.dma_start(out=Wv_sb, in_=W["Wv"].rearrange("(j p) c -> p j c", p=P))
            v3 = v_sb.rearrange("p i (h w) -> p i h w", w=W65)

            def qT_proj(dc, pool):
                for sp in range(NQS):
                    ps = pool.tile([P, QSP], F32, tag="pp", name=f"qps_{dc}_{sp}")
                    for j in range(ND):
                        nc.tensor.matmul(
                            ps,
                            Wq_sb[:, j, dc * P:(dc + 1) * P],
                            xT[:, j, sp * QSP:(sp + 1) * QSP],
                            start=(j == 0), stop=(j == ND - 1),
                        )
                    nc.vector.tensor_scalar_add(
                        qTs[:, dc, sp * QSP:(sp + 1) * QSP], ps, bq_sb[:, dc:dc + 1])

            def v_proj(i, sp, pool):
                if sp == 0:
                    nc.sync.dma_start(out=v3[:, i, :, DH:DH + 1],
                                      in_=ones_d[:, 0:H].unsqueeze(2))
                ps = pool.tile([P, CSP], F32, tag="pp", name=f"vps_{i}_{sp}")
                for j in range(ND):
                    nc.tensor.matmul(
                        ps,
                        xT[:, j, i * P:(i + 1) * P],
                        Wv_sb[:, j, sp * CSP:(sp + 1) * CSP],
                        start=(j == 0), stop=(j == ND - 1),
                    )
                nc.vector.tensor_copy(
                    v3[:, i, sp * HPS:(sp + 1) * HPS, 0:DH],
                    ps.rearrange("p (h w) -> p h w", w=DH),
                )

            Wk_sb = wp.tile([P, ND, D], BF16, tag="w")
            nc.sync.dma_start(out=Wk_sb, in_=W["Wk"].rearrange("(j p) c -> p j c", p=P))

            def kT_proj(p, pool=ppsL):
                kp = kpool.tile([P, S], BF16, tag="kp", name=f"kp_{p}")
                for sp in range(NKS):
                    ps = pool.tile([P, KSP], F32, tag="pp", name=f"kps_{p}_{sp}")
                    for j in range(ND):
                        nc.tensor.matmul(
                            ps,
                            Wk_sb[:, j, p * P:(p + 1) * P],
                            xT[:, j, sp * KSP:(sp + 1) * KSP],
                            start=(j == 0), stop=(j == ND - 1),
                        )
                    nc.vector.tensor_copy(kp[:, sp * KSP:(sp + 1) * KSP], ps)
                return kp

            for dc in range(2):
                qT_proj(dc, ppsE)
            kps = [kT_proj(0, ppsE), kT_proj(1, ppsE)]
            for i in range(NS):
                v_proj(i, 0, ppsE)

        # ---- Attention (kT projection of pair p+2 interleaved) ----
        with tc.tile_pool(name="exp", bufs=4) as exq, \
             tc.tile_pool(name="eps", bufs=4) as eps, \
             tc.tile_pool(name="ystgL", bufs=2) as ystgL, \
             tc.tile_pool(name="scps", bufs=2, space="PSUM") as scps, \
             tc.tile_pool(name="ops", bufs=2, space="PSUM") as opsum:

            def o_chunk(sc_i, spc):
                ps = ppsL.tile([P, CSP], F32, tag="pp",
                               name=f"yps_{sc_i}_{spc}")
                for j in range(ND):
                    nc.tensor.matmul(
                        ps,
                        oT[:, j, sc_i * P:(sc_i + 1) * P],
                        Wo_sb[:, j, spc * CSP:(spc + 1) * CSP],
                        start=(j == 0), stop=(j == ND - 1),
                    )
                ysb = ystgL.tile([P, CSP], F32, tag="ysb")
                nc.vector.tensor_add(
                    ysb, ps, bo_bc[:, spc * CSP:(spc + 1) * CSP])
                nc.sync.dma_start(
                    out=y[sc_i * P:(sc_i + 1) * P, spc * CSP:(spc + 1) * CSP],
                    in_=ysb,
                )

            for p in range(NPAIR):
                kp = kps[p]
                for sp in range(NQS):
                    qsl = slice(sp * QSP, (sp + 1) * QSP)
                    o_even = opsum.tile([W65, QSP], F32, tag="op")
                    o_odd = opsum.tile([W65, QSP], F32, tag="op")
                    for kt in range(NS):
                        if p == NPAIR - 1 and sp == 1 and kt % 2 == 1:
                            # O-projection q-span 0 chunks ride pair 7's
                            # span-1 slack (oT span 0 is fully normalized)
                            ci = (kt - 1) // 2
                            o_chunk(ci // NCS, ci % NCS)
                        sc = scps.tile([P, 2 * QSP], F32, tag="sc")
                        nc.tensor.matmul(
                            sc[:, 0:QSP],
                            kp[0:DH, kt * P:(kt + 1) * P],
                            qTs[0:DH, p, qsl],
                            start=True, stop=True,
                        )
                        nc.tensor.matmul(
                            sc[:, QSP:2 * QSP],
                            kp[DH:P, kt * P:(kt + 1) * P],
                            qTs[DH:P, p, qsl],
                            start=True, stop=True,
                        )
                        ex = exq.tile([P, 2 * QSP], BF16, tag="ex")
                        nc.scalar.activation(ex, sc, AF.Exp, scale=float(scale))
                        nc.tensor.matmul(
                            o_even,
                            v3[:, kt, 2 * p, :],
                            ex[:, 0:QSP],
                            start=(kt == 0), stop=(kt == NS - 1),
                        )
                        nc.tensor.matmul(
                            o_odd,
                            v3[:, kt, 2 * p + 1, :],
                            ex[:, QSP:2 * QSP],
                            start=(kt == 0), stop=(kt == NS - 1),
                        )
                    # epilogue: normalize + bias, assemble oT
                    for par, ops in ((0, o_even), (1, o_odd)):
                        den0 = eps.tile([1, QSP], F32, tag="den0")
                        nc.vector.tensor_copy(den0, ops[DH:W65, :])
                        rc0 = eps.tile([1, QSP], F32, tag="rc0")
                        nc.vector.reciprocal_approx_fast(rc0, den0)
                        rc0b = eps.tile([1, QSP], BF16, tag="rc0b")
                        nc.vector.tensor_copy(rc0b, rc0)
                        rb_ps = ppsL.tile([DH, QSP], F32, tag="rb")
                        nc.tensor.matmul(
                            rb_ps, ones_t, rc0b,
                            start=True, stop=True,
                        )
                        rb = eps.tile([DH, QSP], F32, tag="rb_sb")
                        nc.vector.tensor_copy(rb, rb_ps)
                        if par == 0:
                            nc.vector.tensor_mul(oT[0:DH, p, qsl],
                                                 ops[0:DH, :], rb)
                        else:
                            on = eps.tile([DH, QSP], BF16, tag="on")
                            nc.vector.tensor_mul(on, ops[0:DH, :], rb)
                            nc.sync.dma_start(out=oT[DH:P, p, qsl], in_=on)
                if p + 2 < NPAIR:
                    qT_proj(p + 2, ppsL)
                    kps.append(kT_proj(p + 2))
                if p < 4 and NCS > 1:
                    for i in range(p * NS // 4, (p + 1) * NS // 4):
                        v_proj(i, 1, ppsL)
                if p == NPAIR - 2:
                    # load Wo while pair 7 runs (wp slot freed by kT_proj(7))
                    Wo_sb = wp.tile([P, ND, D], BF16, tag="w", name="Wo")
                    nc.sync.dma_start(
                        out=Wo_sb, in_=W["Wo"].rearrange("(j p) c -> p j c", p=P))

        # ---- Output projection tail: q-span 1 ----
        with tc.tile_pool(name="ystg", bufs=4) as ystg, \
             tc.tile_pool(name="yps", bufs=4, space="PSUM") as yps:
            for sc_i in range(QSP // P, Sq // P):
                for spc in range(NCS):
                    ps = yps.tile([P, CSP], F32, tag="yp")
                    for j in range(ND):
                        nc.tensor.matmul(
                            ps,
                            oT[:, j, sc_i * P:(sc_i + 1) * P],
                            Wo_sb[:, j, spc * CSP:(spc + 1) * CSP],
                            start=(j == 0), stop=(j == ND - 1),
                        )
                    ysb = ystg.tile([P, CSP], F32, tag="ysb")
                    nc.vector.tensor_add(ysb, ps, bo_bc[:, spc * CSP:(spc + 1) * CSP])
                    nc.sync.dma_start(
                        out=y[sc_i * P:(sc_i + 1) * P, spc * CSP:(spc + 1) * CSP],
                        in_=ysb,
                    )

    nc.compile()
    return nc


_NC = None


def _get_nc():
    global _NC
    if _NC is None:
        _NC = build_mha_nc(S=S_FULL, Sq=S_FULL // 2, D=D_FULL, H=H_FULL)
    return _NC


def shard_inputs(inputs):
    x = np.asarray(inputs["x"], dtype=np.float32).astype(NPBF16)
    wnames = ("Wq", "Wk", "Wv", "Wo")
    shared = {n: np.ascontiguousarray(
        np.asarray(inputs[n], dtype=np.float32).astype(NPBF16)) for n in wnames}
    shared["bq"] = np.ascontiguousarray(np.asarray(inputs["bq"], dtype=np.float32))
    # bv contributes bv @ Wo to y (attention rows sum to 1); fold into bo
    bv = np.asarray(inputs["bv"], dtype=np.float32)
    Wo = np.asarray(inputs["Wo"], dtype=np.float32)
    bo = np.asarray(inputs["bo"], dtype=np.float32)
    shared["bo"] = np.ascontiguousarray(bo + bv @ Wo)
    shared["cst_ones"] = np.ones((P, P), dtype=NPBF16)
    half = S_FULL // 2
    maps = []
    for c in range(N_CORES):
        b, h = divmod(c, 2)
        xb = x[b]
        xp = np.concatenate([xb[h * half:(h + 1) * half],
                             xb[(1 - h) * half:(2 - h) * half]], axis=0)
        m = dict(shared)
        m["x"] = np.ascontiguousarray(xp)
        maps.append(m)
    return maps


def run(inputs, trace=False):
    nc = _get_nc()
    maps = shard_inputs(inputs)
    res = run_bass_kernel_spmd(nc, maps, list(range(N_CORES)), trace=trace)
    half = S_FULL // 2
    y = np.empty((B_FULL, S_FULL, D_FULL), dtype=np.float32)
    for c in range(N_CORES):
        b, h = divmod(c, 2)
        y[b, h * half:(h + 1) * half] = res.results[c]["y"]
    return y, res


def kernel(**inputs):
    y, _ = run(inputs, trace=False)
    return y



# revision 20
# speedup vs baseline: 1.2580x; 1.0036x over previous
"""Trainium2 Bass kernel for nn_MultiHeadAttention (B=4, S=2048, D=1024, H=16).

Sharding: 8 cores = 4 batches x 2 query-halves. Each core computes full K/V
projections for its batch (keys are permuted so the core's own queries come
first), attention for its 1024 queries over all 2048 keys, and the output
projection for its query half. No collectives needed.

Per-core dataflow (bf16 matmuls, fp32 PSUM accumulation, all tensors SBUF
resident — no DRAM spills):
  x [2048,1024] bf16 --PE transpose--> xT [D,S]
  qT = (x @ Wq)^T [D,1024] and v_aug = [x @ Wv | ones] computed up front;
  kT d-tiles are projected per head-pair, interleaved into the attention
  loop so the PE fills the gaps of the ACT(exp)-paced inner loop.
  Per head pair p, per q-span of 512:
    scoresT[k,q] tiles via row-paired (tile_position) K=64 matmuls
    exp on ACT (scale=1/8 folded in), flash-style, no max subtraction
    outT[65,q] accumulated in PSUM via v_aug=[v_h | ones] stationary
    normalize by row 64 (denominator via reciprocal_approx_fast +
    K=1 fp32 PE broadcast), add bv, assemble oT [D, Sq]
  y = oT^T @ Wo + bo  -> [1024, 1024] fp32
"""

import os
import numpy as np
import ml_dtypes
from contextlib import ExitStack

import concourse.bass as bass
from concourse import bacc
import concourse.mybir as mybir
import concourse.tile as tile
from concourse.bass_utils import run_bass_kernel_spmd
from concourse.masks import make_identity

F32 = mybir.dt.float32
BF16 = mybir.dt.bfloat16
AF = mybir.ActivationFunctionType
NPBF16 = ml_dtypes.bfloat16

P = 128

N_CORES = 8
B_FULL, S_FULL, D_FULL = 4, 2048, 1024
H_FULL, DH = 16, 64


def build_mha_nc(S=2048, Sq=1024, D=1024, H=16, scale=None):
    """Build the per-core Bass program. Returns nc."""
    assert D % P == 0 and S % P == 0 and Sq % P == 0 and H % 2 == 0
    ND = D // P            # d-tiles
    NS = S // P            # s-chunks / k-tiles
    NPAIR = H // 2
    W65 = DH + 1           # augmented head width (v | ones)
    QSP = min(512, Sq)     # q span
    NQS = Sq // QSP
    KSP = min(512, S)      # span for kT projection
    NKS = S // KSP
    CSP = min(512, D)      # col span for v / out projections
    NCS = D // CSP
    HPS = CSP // DH        # heads per col-span in v projection
    if scale is None:
        scale = DH ** -0.5

    nc = bacc.Bacc(target_bir_lowering=False, debug=False)

    x = nc.dram_tensor("x", [S, D], BF16, kind="ExternalInput").ap()
    W = {n: nc.dram_tensor(n, [D, D], BF16, kind="ExternalInput").ap()
         for n in ("Wq", "Wk", "Wv", "Wo")}
    bias = {n: nc.dram_tensor(n, [D], F32, kind="ExternalInput").ap()
            for n in ("bq", "bk", "bv", "bo")}
    ones_d = nc.dram_tensor("cst_ones", [P, P], BF16, kind="ExternalInput").ap()
    y = nc.dram_tensor("y", [Sq, D], F32, kind="ExternalOutput").ap()

    with tile.TileContext(nc) as tc, ExitStack() as top:
        top.enter_context(nc.allow_low_precision(
            reason="bf16 activations/weights with fp32 psum accumulation"))
        const = top.enter_context(tc.tile_pool(name="const", bufs=1))
        big = top.enter_context(tc.tile_pool(name="big", bufs=1))
        wp = top.enter_context(tc.tile_pool(name="wp", bufs=2))
        kpool = top.enter_context(tc.tile_pool(name="kpool", bufs=3))
        ppsL = top.enter_context(tc.tile_pool(name="ppsL", bufs=1, space="PSUM"))

        ident = const.tile([P, P], BF16)
        make_identity(nc, ident)
        # bf16 ones row: K=1 stationary broadcasting the softmax denominator
        ones_t = const.tile([1, DH], BF16)
        nc.vector.memset(ones_t, 1.0)

        # per-partition bias layouts: b_sb[p, j] = b[j*128 + p]
        bq_sb = const.tile([P, ND], F32)
        nc.gpsimd.dma_start(out=bq_sb, in_=bias["bq"].rearrange("(j p) -> p j", p=P))
        bk_sb = const.tile([P, ND], F32)
        nc.gpsimd.dma_start(out=bk_sb, in_=bias["bk"].rearrange("(j p) -> p j", p=P))
        # bv split by head parity within a d-tile (used at base partition 0)
        bv_even = const.tile([DH, ND], F32)
        nc.gpsimd.dma_start(
            out=bv_even,
            in_=bias["bv"].rearrange("(j q p) -> q p j", p=DH, q=2)[0],
        )
        bv_odd = const.tile([DH, ND], F32)
        nc.gpsimd.dma_start(
            out=bv_odd,
            in_=bias["bv"].rearrange("(j q p) -> q p j", p=DH, q=2)[1],
        )
        # bo broadcast across partitions (0-stride DRAM read)
        bo_bc = const.tile([P, D], F32)
        nc.gpsimd.dma_start(
            out=bo_bc,
            in_=bias["bo"].unsqueeze(0).partition_broadcast(P).squeeze(1),
        )

        oT = big.tile([P, ND, Sq], BF16)
        xT = big.tile([P, ND, S], BF16)
        qTs = big.tile([P, ND, Sq], BF16)
        v_sb = big.tile([P, NS, H * W65], BF16)

        # ---- Phase T: PE-transpose x into xT ----
        with tc.tile_pool(name="xchunk", bufs=3) as xpool, \
             tc.tile_pool(name="tps", bufs=2, space="PSUM") as tpsum, \
             tc.tile_pool(name="ppsE", bufs=4, space="PSUM") as ppsE:
            for i in range(NS):
                xc = xpool.tile([P, D], BF16, tag="xc")
                nc.sync.dma_start(out=xc, in_=x[i * P:(i + 1) * P, :])
                for j in range(ND):
                    tp = tpsum.tile([P, P], BF16, tag="tp")
                    nc.tensor.transpose(tp, xc[:, j * P:(j + 1) * P], ident)
                    nc.vector.tensor_copy(xT[:, j, i * P:(i + 1) * P], tp)

            # ---- qT / v projections: helpers; early part emits only what
            # pair 0 needs, the rest interleaves into the attention loop ----
            Wq_sb = wp.tile([P, ND, D], BF16, tag="w")
            nc.sync.dma_start(out=Wq_sb, in_=W["Wq"].rearrange("(j p) c -> p j c", p=P))
            Wv_sb = wp.tile([P, ND, D], BF16, tag="wv", bufs=1)
            nc.sync.dma_start(out=Wv_sb, in_=W["Wv"].rearrange("(j p) c -> p j c", p=P))
            v3 = v_sb.rearrange("p i (h w) -> p i h w", w=W65)

            def qT_proj(dc, pool):
                for sp in range(NQS):
                    ps = pool.tile([P, QSP], F32, tag="pp", name=f"qps_{dc}_{sp}")
                    for j in range(ND):
                        nc.tensor.matmul(
                            ps,
                            Wq_sb[:, j, dc * P:(dc + 1) * P],
                            xT[:, j, sp * QSP:(sp + 1) * QSP],
                            start=(j == 0), stop=(j == ND - 1),
                        )
                    nc.vector.tensor_scalar_add(
                        qTs[:, dc, sp * QSP:(sp + 1) * QSP], ps, bq_sb[:, dc:dc + 1])

            def v_proj(i, sp, pool):
                if sp == 0:
                    nc.sync.dma_start(out=v3[:, i, :, DH:DH + 1],
                                      in_=ones_d[:, 0:H].unsqueeze(2))
                ps = pool.tile([P, CSP], F32, tag="pp", name=f"vps_{i}_{sp}")
                for j in range(ND):
                    nc.tensor.matmul(
                        ps,
                        xT[:, j, i * P:(i + 1) * P],
                        Wv_sb[:, j, sp * CSP:(sp + 1) * CSP],
                        start=(j == 0), stop=(j == ND - 1),
                    )
                nc.vector.tensor_copy(
                    v3[:, i, sp * HPS:(sp + 1) * HPS, 0:DH],
                    ps.rearrange("p (h w) -> p h w", w=DH),
                )

            Wk_sb = wp.tile([P, ND, D], BF16, tag="w")
            nc.sync.dma_start(out=Wk_sb, in_=W["Wk"].rearrange("(j p) c -> p j c", p=P))

            def kT_proj(p, pool=ppsL):
                kp = kpool.tile([P, S], BF16, tag="kp", name=f"kp_{p}")
                for sp in range(NKS):
                    ps = pool.tile([P, KSP], F32, tag="pp", name=f"kps_{p}_{sp}")
                    for j in range(ND):
                        nc.tensor.matmul(
                            ps,
                            Wk_sb[:, j, p * P:(p + 1) * P],
                            xT[:, j, sp * KSP:(sp + 1) * KSP],
                            start=(j == 0), stop=(j == ND - 1),
                        )
                    nc.vector.tensor_scalar_add(
                        kp[:, sp * KSP:(sp + 1) * KSP], ps, bk_sb[:, p:p + 1])
                return kp

            for dc in range(2):
                qT_proj(dc, ppsE)
            kps = [kT_proj(0, ppsE), kT_proj(1, ppsE)]
            for i in range(NS):
                v_proj(i, 0, ppsE)

        # ---- Attention (kT projection of pair p+2 interleaved) ----
        with tc.tile_pool(name="exp", bufs=4) as exq, \
             tc.tile_pool(name="eps", bufs=4) as eps, \
             tc.tile_pool(name="scps", bufs=2, space="PSUM") as scps, \
             tc.tile_pool(name="ops", bufs=2, space="PSUM") as opsum:
            for p in range(NPAIR):
                kp = kps[p]
                for sp in range(NQS):
                    qsl = slice(sp * QSP, (sp + 1) * QSP)
                    o_even = opsum.tile([W65, QSP], F32, tag="op")
                    o_odd = opsum.tile([W65, QSP], F32, tag="op")
                    for kt in range(NS):
                        sc = scps.tile([P, 2 * QSP], F32, tag="sc")
                        nc.tensor.matmul(
                            sc[:, 0:QSP],
                            kp[0:DH, kt * P:(kt + 1) * P],
                            qTs[0:DH, p, qsl],
                            start=True, stop=True,
                        )
                        nc.tensor.matmul(
                            sc[:, QSP:2 * QSP],
                            kp[DH:P, kt * P:(kt + 1) * P],
                            qTs[DH:P, p, qsl],
                            start=True, stop=True,
                        )
                        ex = exq.tile([P, 2 * QSP], BF16, tag="ex")
                        nc.scalar.activation(ex, sc, AF.Exp, scale=float(scale))
                        nc.tensor.matmul(
                            o_even,
                            v3[:, kt, 2 * p, :],
                            ex[:, 0:QSP],
                            start=(kt == 0), stop=(kt == NS - 1),
                        )
                        nc.tensor.matmul(
                            o_odd,
                            v3[:, kt, 2 * p + 1, :],
                            ex[:, QSP:2 * QSP],
                            start=(kt == 0), stop=(kt == NS - 1),
                        )
                    # epilogue: normalize + bias, assemble oT
                    for par, ops in ((0, o_even), (1, o_odd)):
                        den0 = eps.tile([1, QSP], F32, tag="den0")
                        nc.vector.tensor_copy(den0, ops[DH:W65, :])
                        rc0 = eps.tile([1, QSP], F32, tag="rc0")
                        nc.vector.reciprocal_approx_fast(rc0, den0)
                        rc0b = eps.tile([1, QSP], BF16, tag="rc0b")
                        nc.vector.tensor_copy(rc0b, rc0)
                        rb_ps = ppsL.tile([DH, QSP], F32, tag="rb")
                        nc.tensor.matmul(
                            rb_ps, ones_t, rc0b,
                            start=True, stop=True,
                        )
                        rb = eps.tile([DH, QSP], F32, tag="rb_sb")
                        nc.vector.tensor_copy(rb, rb_ps)
                        bv_sb = bv_even if par == 0 else bv_odd
                        if par == 0:
                            dst = oT[0:DH, p, qsl]
                            nc.vector.tensor_mul(dst, ops[0:DH, :], rb)
                            nc.vector.tensor_scalar_add(dst, dst, bv_sb[:, p:p + 1])
                        else:
                            on = eps.tile([DH, QSP], BF16, tag="on")
                            nc.vector.tensor_mul(on, ops[0:DH, :], rb)
                            nc.vector.tensor_scalar_add(on, on, bv_sb[:, p:p + 1])
                            nc.sync.dma_start(out=oT[DH:P, p, qsl], in_=on)
                if p + 2 < NPAIR:
                    qT_proj(p + 2, ppsL)
                    kps.append(kT_proj(p + 2))
                if p < 4 and NCS > 1:
                    for i in range(p * NS // 4, (p + 1) * NS // 4):
                        v_proj(i, 1, ppsL)

        # ---- Output projection ----
        Wo_sb = wp.tile([P, ND, D], BF16, tag="w")
        nc.sync.dma_start(out=Wo_sb, in_=W["Wo"].rearrange("(j p) c -> p j c", p=P))
        with tc.tile_pool(name="ystg", bufs=4) as ystg, \
             tc.tile_pool(name="yps", bufs=4, space="PSUM") as yps:
            for sc_i in range(Sq // P):
                for sp in range(NCS):
                    ps = yps.tile([P, CSP], F32, tag="yp")
                    for j in range(ND):
                        nc.tensor.matmul(
                            ps,
                            oT[:, j, sc_i * P:(sc_i + 1) * P],
                            Wo_sb[:, j, sp * CSP:(sp + 1) * CSP],
                            start=(j == 0), stop=(j == ND - 1),
                        )
                    ysb = ystg.tile([P, CSP], F32, tag="ysb")
                    nc.vector.tensor_add(ysb, ps, bo_bc[:, sp * CSP:(sp + 1) * CSP])
                    nc.sync.dma_start(
                        out=y[sc_i * P:(sc_i + 1) * P, sp * CSP:(sp + 1) * CSP],
                        in_=ysb,
                    )

    nc.compile()
    return nc


_NC = None


def _get_nc():
    global _NC
    if _NC is None:
        _NC = build_mha_nc(S=S_FULL, Sq=S_FULL // 2, D=D_FULL, H=H_FULL)
    return _NC


def shard_inputs(inputs):
    x = np.asarray(inputs["x"], dtype=np.float32).astype(NPBF16)
    wnames = ("Wq", "Wk", "Wv", "Wo")
    bnames = ("bq", "bk", "bv", "bo")
    shared = {n: np.ascontiguousarray(
        np.asarray(inputs[n], dtype=np.float32).astype(NPBF16)) for n in wnames}
    shared.update({n: np.ascontiguousarray(np.asarray(inputs[n], dtype=np.float32))
                   for n in bnames})
    shared["cst_ones"] = np.ones((P, P), dtype=NPBF16)
    half = S_FULL // 2
    maps = []
    for c in range(N_CORES):
        b, h = divmod(c, 2)
        xb = x[b]
        xp = np.concatenate([xb[h * half:(h + 1) * half],
                             xb[(1 - h) * half:(2 - h) * half]], axis=0)
        m = dict(shared)
        m["x"] = np.ascontiguousarray(xp)
        maps.append(m)
    return maps


def run(inputs, trace=False):
    nc = _get_nc()
    maps = shard_inputs(inputs)
    res = run_bass_kernel_spmd(nc, maps, list(range(N_CORES)), trace=trace)
    half = S_FULL // 2
    y = np.empty((B_FULL, S_FULL, D_FULL), dtype=np.float32)
    for c in range(N_CORES):
        b, h = divmod(c, 2)
        y[b, h * half:(h + 1) * half] = res.results[c]["y"]
    return y, res


def kernel(**inputs):
    y, _ = run(inputs, trace=False)
    return y

